# revision 20
# baseline (speedup 1.0000x reference)
"""Trainium2 Bass kernel for nn_LongRangeDW (dense_cnn).

The module is entirely linear in x:
  s = nnstacking(x)                        (5 shifted copies, clipped to window)
  y = dw1(s) + dw2(s) + dw3(s)             (depthwise 1x1 + 3x3 d8 + 3x3 d12)
  out = pw(y) + x                          (pointwise 5C->C + residual)

Folding everything gives
  out[o, p] = sum_{g,t} (W4_g diag(k_{g,t}))[o,:] @ xe[:, p + tau_t + sigma_g]
              + beff[o] + x[o, p]
with xe = zero-extended x: 85 distinct offsets -> 85 precomputed 128x128
matrices applied to shifted views of a zero-padded SBUF-resident image, all
accumulated in PSUM by the tensor engine (float32r: 1 col/cycle).

The composition of clipped shifts with zero-padded convs is NOT the padded
composite: where a depthwise tap lands exactly 1 px outside the window and the
nnstacking shift sigma_g pulls it back in, the composite wrongly reads x. The
mismatch lives on 8 one-pixel strips (output rows/cols {7,11,116,120}) reading
x's 4 border lines -> 24 extra small matmuls, folded in during evacuation.

Data parallel: batch B=8 -> one image per NeuronCore.
"""

import os
import sys

import numpy as np

sys.path.insert(0, "/opt/trn_rl_repo")

B, C, H, W = 8, 128, 128, 128
PAD = 13            # max |offset| = 12 + 1
HP = H + 2 * PAD    # 154
WP = W + 2 * PAD
N_CORES = 8
SB_ROWS = 16        # output rows per super-block (psum tile = 4 banks = 2048 fp32)
N_SB = H // SB_ROWS
SUB_ROWS = 4        # rows per matmul (N = 512 moving limit for 4-byte dtypes)

SHIFTS = [(1, 0), (-1, 0), (0, 1), (0, -1), (0, 0)]  # nnstacking groups


# --------------------------------------------------------------------------
# host-side operator folding
# --------------------------------------------------------------------------

def _build_terms(w1, w2, w3, w4):
    """85 main terms: list of ((di, dj), M[o, c]) in fixed (sorted) order.

    NOTE: the residual identity is NOT folded in (weights are bf16 on device;
    the residual is added exactly in fp32 during evacuation instead)."""
    w4m = w4[:, :, 0, 0].astype(np.float64)  # [C, 5C]
    taps = {}
    for g in range(5):
        sy, sx = SHIFTS[g]
        sl = slice(g * C, (g + 1) * C)

        def add(di, dj, kv, sl=sl, g=g):
            M = taps.setdefault((di, dj), np.zeros((C, C), np.float64))
            M += w4m[:, sl] * kv.astype(np.float64)[None, :]

        add(sy, sx, w1[sl, 0, 0, 0])
        for w, d in ((w2, 8), (w3, 12)):
            for a in range(3):
                for b in range(3):
                    add(sy + (a - 1) * d, sx + (b - 1) * d, w[sl, 0, a, b])
    offsets = sorted(taps)
    mats = [taps[off] for off in offsets]
    return offsets, np.stack(mats).astype(np.float32)


def _build_corrections(w2, w3, w4):
    """24 strip-correction terms (matrices already NEGATED for accumulation).

    Strips (j = 0..7):
      j<4:  column strips: out col px, reading x col src, row shift ty
      j>=4: row strips:    out row py, reading x row src, col shift tx
    Each strip has 3 taps. Returns (strips, mats[24, C, C]).
    strips: list of dict(kind, fixed_out, src, shifts[3])
    """
    w4m = w4[:, :, 0, 0].astype(np.float64)
    strips, mats = [], []
    # (kind, group g, weight tensor, dilation)
    specs = [
        ("col", 2, 8),   # px = -1 - (-8) = 7,  src col 0
        ("col", 2, 12),  # px = 11,             src col 0
        ("col", 3, 12),  # px = 128 - 12 = 116, src col 127
        ("col", 3, 8),   # px = 120,            src col 127
        ("row", 0, 8),   # py = 7,   src row 0
        ("row", 0, 12),  # py = 11,  src row 0
        ("row", 1, 12),  # py = 116, src row 127
        ("row", 1, 8),   # py = 120, src row 127
    ]
    for kind, g, d in specs:
        sy, sx = SHIFTS[g]
        sl = slice(g * C, (g + 1) * C)
        w = w2 if d == 8 else w3
        if kind == "col":
            border = -1 if sx == 1 else W          # (p+tau)_x
            fixed_out = border - (-d if sx == 1 else d)
            src = border + sx                       # x col actually read
            shifts = [-d, 0, d]                     # ty values
            tap_b = 0 if sx == 1 else 2             # b index with tx = -d / +d
            kvs = [w[sl, 0, a, tap_b] for a in range(3)]
        else:
            border = -1 if sy == 1 else H
            fixed_out = border - (-d if sy == 1 else d)
            src = border + sy
            shifts = [-d, 0, d]                     # tx values
            tap_a = 0 if sy == 1 else 2
            kvs = [w[sl, 0, tap_a, b] for b in range(3)]
        strips.append(dict(kind=kind, fixed_out=fixed_out, src=src, shifts=shifts))
        for kv in kvs:
            mats.append(-(w4m[:, sl] * kv.astype(np.float64)[None, :]))
    return strips, np.stack(mats).astype(np.float32)


def _build_weights(inputs):
    """Returns wt [C, 109*C] fp32 (lhsT layout: wt[c, blk*C + o] = M_blk[o, c]),
    beff [C] fp32, offsets, strips."""
    w1, w2, w3, w4 = inputs["w1"], inputs["w2"], inputs["w3"], inputs["w4"]
    b1, b2, b3, b4 = inputs["b1"], inputs["b2"], inputs["b3"], inputs["b4"]
    offsets, mats = _build_terms(w1, w2, w3, w4)
    strips, cmats = _build_corrections(w2, w3, w4)
    allm = np.concatenate([mats, cmats], axis=0)          # [109, C(o), C(c)]
    wt = np.ascontiguousarray(allm.transpose(2, 0, 1).reshape(C, -1)).astype(np.float32)
    w4m = w4[:, :, 0, 0].astype(np.float64)
    beff = (b4.astype(np.float64)
            + w4m @ (b1 + b2 + b3).astype(np.float64)).astype(np.float32)
    return wt, beff, offsets, strips


# --------------------------------------------------------------------------
# device program
# --------------------------------------------------------------------------

_CACHE = {}


def _build_program(offsets, strips):
    import concourse.bacc as bacc
    import concourse.mybir as mybir
    import concourse.tile as tile

    nc = bacc.Bacc("TRN2", target_bir_lowering=False)
    f32 = mybir.dt.float32
    f32r = mybir.dt.float32r

    bf16 = mybir.dt.bfloat16
    n_blk = 85 + 24
    # Weights are bf16: fp32/f32r weights force a self-loading matmul which
    # has a single sync-wait slot and breaks walrus codegen under Tile's
    # slot-reuse waits. bf16 weights use the standard LDWEIGHTS+MATMUL split.
    xp_d = nc.dram_tensor("xp", [C, HP * WP], bf16, kind="ExternalInput")
    wt_d = nc.dram_tensor("wt", [C, n_blk * C], bf16, kind="ExternalInput")
    xres_d = nc.dram_tensor("xres", [C, H * W], f32, kind="ExternalInput")
    beff_d = nc.dram_tensor("beff", [C, 1], f32, kind="ExternalInput")
    out_d = nc.dram_tensor("out", [C, H * W], f32, kind="ExternalOutput")

    with tile.TileContext(nc) as tc:
        with (
            tc.tile_pool(name="const", bufs=1) as const,
            tc.tile_pool(name="outp", bufs=3) as outp,
            tc.tile_pool(name="psum", bufs=2, space="PSUM") as psum_pool,
        ):
            xp_sb = const.tile([C, HP * WP], bf16)
            wt_sb = const.tile([C, n_blk * C], bf16)
            xres_sb = const.tile([C, H * W], f32)
            beff_sb = const.tile([C, 1], f32)

            # DMA strategy: SWDGE (nc.gpsimd) fans >=1MB transfers across all
            # 16 SDMA engines (~340 GB/s); HWDGE chunks ran at ~26 GB/s on a
            # single engine. Order: minimum needed for SB0 first (wt chunk 0 +
            # first xp rows), then the rest.
            WT_CHUNK = 55 * C
            nc.gpsimd.dma_start(out=wt_sb[:, :WT_CHUNK], in_=wt_d[:, :WT_CHUNK])
            ROWS0 = SB_ROWS + 2 * PAD  # padded rows needed by SB0
            nc.gpsimd.dma_start(out=xp_sb[:, :ROWS0 * WP],
                                in_=xp_d[:, :ROWS0 * WP])
            nc.gpsimd.dma_start(out=wt_sb[:, WT_CHUNK:], in_=wt_d[:, WT_CHUNK:])
            XP_CHUNK_ROWS = 56
            for r0_ in range(ROWS0, HP, XP_CHUNK_ROWS):
                r1_ = min(r0_ + XP_CHUNK_ROWS, HP)
                nc.gpsimd.dma_start(out=xp_sb[:, r0_ * WP:r1_ * WP],
                                    in_=xp_d[:, r0_ * WP:r1_ * WP])
            nc.sync.dma_start(out=beff_sb, in_=beff_d[:, :])
            for q0 in range(0, H * W, H * W // 4):
                q1 = q0 + H * W // 4
                nc.gpsimd.dma_start(out=xres_sb[:, q0:q1], in_=xres_d[:, q0:q1])

            xp3 = xp_sb.rearrange("p (r w) -> p r w", w=WP)

            def wblk(i):
                return wt_sb[:, i * C:(i + 1) * C]

            corr_sb = const.tile([C, 8 * H], f32)

            def emit_corrections():
                # needs the full xp image -> emitted after SB0's matmuls
                psum_c_full = psum_pool.tile([C, SB_ROWS * W], f32, tag="acc",
                                             name="psum_c")
                psum_c = psum_c_full[:, :8 * H]
                for j, st in enumerate(strips):
                    for i, sh in enumerate(st["shifts"]):
                        if st["kind"] == "col":
                            # out rows 0..127 at fixed col; x col src, rows r+ty
                            rhs = xp3[:, PAD + sh: PAD + sh + H,
                                      PAD + st["src"]: PAD + st["src"] + 1]
                        else:
                            # out cols 0..127 at fixed row; x row src, cols j+tx
                            rhs = xp3[:, PAD + st["src"]: PAD + st["src"] + 1,
                                      PAD + sh: PAD + sh + W]
                        nc.tensor.matmul(psum_c[:, j * H:(j + 1) * H],
                                         wblk(85 + 3 * j + i), rhs,
                                         start=(i == 0), stop=(i == 2))
                nc.vector.tensor_copy(corr_sb, psum_c)

            # ---- main loop -------------------------------------------------
            n_sub = SB_ROWS // SUB_ROWS
            for s in range(N_SB):
                r0 = s * SB_ROWS
                psum = psum_pool.tile([C, SB_ROWS * W], f32, tag="acc")
                for t, (di, dj) in enumerate(offsets):
                    for u in range(n_sub):
                        a0 = PAD + r0 + u * SUB_ROWS + di
                        rhs = xp3[:, a0: a0 + SUB_ROWS, PAD + dj: PAD + dj + W]
                        nc.tensor.matmul(
                            psum[:, u * SUB_ROWS * W:(u + 1) * SUB_ROWS * W],
                            wblk(t), rhs,
                            start=(t == 0), stop=(t == len(offsets) - 1))

                if s == 0:
                    emit_corrections()

                out_sb = outp.tile([C, SB_ROWS * W], f32)
                nc.scalar.activation(out_sb, psum,
                                     mybir.ActivationFunctionType.Identity,
                                     bias=beff_sb[:, 0:1])
                nc.vector.tensor_add(
                    out_sb, out_sb,
                    xres_sb[:, r0 * W:(r0 + SB_ROWS) * W])
                out3 = out_sb.rearrange("p (r w) -> p r w", w=W)
                for j, st in enumerate(strips):
                    if st["kind"] == "col":
                        dst = out3[:, 0:SB_ROWS, st["fixed_out"]:st["fixed_out"] + 1]
                        src = corr_sb[:, j * H + r0: j * H + r0 + SB_ROWS]
                        nc.vector.tensor_add(dst, dst, src)
                    elif r0 <= st["fixed_out"] < r0 + SB_ROWS:
                        lr = st["fixed_out"] - r0
                        dst = out3[:, lr:lr + 1, :]
                        src = corr_sb[:, j * H: j * H + W]
                        nc.vector.tensor_add(dst, dst, src)
                nc.gpsimd.dma_start(out=out_d[:, r0 * W:(r0 + SB_ROWS) * W],
                                    in_=out_sb)
    nc.finalize()
    return nc


def _make_in_maps(inputs):
    x = np.ascontiguousarray(inputs["x"], dtype=np.float32)
    wt, beff, offsets, strips = _build_weights(inputs)
    if "nc" not in _CACHE:
        _CACHE["nc"] = _build_program(offsets, strips)

    import ml_dtypes
    bf = ml_dtypes.bfloat16
    xpad = np.zeros((B, C, HP, WP), bf)
    xpad[:, :, PAD:PAD + H, PAD:PAD + W] = x.astype(bf)
    beff_col = np.ascontiguousarray(beff.reshape(C, 1))
    wt_bf16 = wt.astype(bf)
    return [
        {
            "xp": np.ascontiguousarray(xpad[b].reshape(C, HP * WP)),
            "wt": wt_bf16,
            "xres": np.ascontiguousarray(x[b].reshape(C, H * W)),
            "beff": beff_col,
        }
        for b in range(B)
    ]


def kernel(**inputs):
    in_maps = _make_in_maps(inputs)
    from concourse.bass_utils import run_bass_kernel_spmd
    res = run_bass_kernel_spmd(_CACHE["nc"], in_maps, core_ids=list(range(N_CORES)))
    out = np.stack([res.results[b]["out"].reshape(C, H, W) for b in range(B)])
    return out.astype(np.float32)


# revision 23
# speedup vs baseline: 1.2037x; 1.2037x over previous
"""Trainium2 Bass kernel for nn_LongRangeDW (dense_cnn).

The module is entirely linear in x:
  s = nnstacking(x)                        (5 shifted copies, clipped to window)
  y = dw1(s) + dw2(s) + dw3(s)             (depthwise 1x1 + 3x3 d8 + 3x3 d12)
  out = pw(y) + x                          (pointwise 5C->C + residual)

Folding everything gives
  out[o, p] = sum_{g,t} (W4_g diag(k_{g,t}))[o,:] @ xe[:, p + tau_t + sigma_g]
              + beff[o] + x[o, p]
with xe = zero-extended x: 85 distinct offsets -> 85 precomputed 128x128
matrices applied to shifted views of a zero-padded SBUF-resident image, all
accumulated in PSUM by the tensor engine (float32r: 1 col/cycle).

The composition of clipped shifts with zero-padded convs is NOT the padded
composite: where a depthwise tap lands exactly 1 px outside the window and the
nnstacking shift sigma_g pulls it back in, the composite wrongly reads x. The
mismatch lives on 8 one-pixel strips (output rows/cols {7,11,116,120}) reading
x's 4 border lines -> 24 extra small matmuls, folded in during evacuation.

Data parallel: batch B=8 -> one image per NeuronCore.
"""

import os
import sys

import numpy as np

sys.path.insert(0, "/opt/trn_rl_repo")

B, C, H, W = 8, 128, 128, 128
PAD = 14            # max |offset| = 12 + 1, rounded even for DVE 4B alignment
HP = H + 2 * PAD    # 154
WP = W + 2 * PAD
N_CORES = 8
SB_ROWS = 16        # output rows per super-block (psum tile = 4 banks = 2048 fp32)
N_SB = H // SB_ROWS
SUB_ROWS = 4        # rows per matmul (N = 512 moving limit for 4-byte dtypes)

SHIFTS = [(1, 0), (-1, 0), (0, 1), (0, -1), (0, 0)]  # nnstacking groups


# --------------------------------------------------------------------------
# host-side operator folding
# --------------------------------------------------------------------------

def _build_terms(w1, w2, w3, w4):
    """Main offset terms for the shifted groups g=0..3 (68 matrices), plus the
    center group g=4 expressed as per-channel taps k4 (computed on the vector
    engine) followed by one pointwise matmul with W4_4.

    Group-4 offsets (both coords even) are disjoint from groups 0-3 (exactly
    one odd coord), so the split removes 17 whole matmul terms.

    NOTE: the residual identity is NOT folded in (weights are bf16 on device;
    the residual is added exactly in fp32 during evacuation instead)."""
    w4m = w4[:, :, 0, 0].astype(np.float64)  # [C, 5C]
    taps = {}
    k4 = {}
    for g in range(5):
        sy, sx = SHIFTS[g]
        sl = slice(g * C, (g + 1) * C)

        def add(di, dj, kv, sl=sl, g=g):
            if g == 4:
                v = k4.setdefault((di, dj), np.zeros(C, np.float64))
                v += kv.astype(np.float64)
            else:
                M = taps.setdefault((di, dj), np.zeros((C, C), np.float64))
                M += w4m[:, sl] * kv.astype(np.float64)[None, :]

        add(sy, sx, w1[sl, 0, 0, 0])
        for w, d in ((w2, 8), (w3, 12)):
            for a in range(3):
                for b in range(3):
                    add(sy + (a - 1) * d, sx + (b - 1) * d, w[sl, 0, a, b])
    offsets = sorted(taps)
    mats = [taps[off] for off in offsets]
    k4_offsets = sorted(k4)
    k4_mat = np.stack([k4[o] for o in k4_offsets], axis=1)  # [C, 17]
    w4c = w4m[:, 4 * C:5 * C]                               # y4 pointwise
    return (offsets, np.stack(mats).astype(np.float32),
            k4_offsets, k4_mat.astype(np.float32), w4c.astype(np.float32))


def _build_corrections(w2, w3, w4):
    """24 strip-correction terms (matrices already NEGATED for accumulation).

    Strips (j = 0..7):
      j<4:  column strips: out col px, reading x col src, row shift ty
      j>=4: row strips:    out row py, reading x row src, col shift tx
    Each strip has 3 taps. Returns (strips, mats[24, C, C]).
    strips: list of dict(kind, fixed_out, src, shifts[3])
    """
    w4m = w4[:, :, 0, 0].astype(np.float64)
    strips, mats = [], []
    # (kind, group g, weight tensor, dilation)
    specs = [
        ("col", 2, 8),   # px = -1 - (-8) = 7,  src col 0
        ("col", 2, 12),  # px = 11,             src col 0
        ("col", 3, 12),  # px = 128 - 12 = 116, src col 127
        ("col", 3, 8),   # px = 120,            src col 127
        ("row", 0, 8),   # py = 7,   src row 0
        ("row", 0, 12),  # py = 11,  src row 0
        ("row", 1, 12),  # py = 116, src row 127
        ("row", 1, 8),   # py = 120, src row 127
    ]
    for kind, g, d in specs:
        sy, sx = SHIFTS[g]
        sl = slice(g * C, (g + 1) * C)
        w = w2 if d == 8 else w3
        if kind == "col":
            border = -1 if sx == 1 else W          # (p+tau)_x
            fixed_out = border - (-d if sx == 1 else d)
            src = border + sx                       # x col actually read
            shifts = [-d, 0, d]                     # ty values
            tap_b = 0 if sx == 1 else 2             # b index with tx = -d / +d
            kvs = [w[sl, 0, a, tap_b] for a in range(3)]
        else:
            border = -1 if sy == 1 else H
            fixed_out = border - (-d if sy == 1 else d)
            src = border + sy
            shifts = [-d, 0, d]                     # tx values
            tap_a = 0 if sy == 1 else 2
            kvs = [w[sl, 0, tap_a, b] for b in range(3)]
        strips.append(dict(kind=kind, fixed_out=fixed_out, src=src, shifts=shifts))
        for kv in kvs:
            mats.append(-(w4m[:, sl] * kv.astype(np.float64)[None, :]))
    return strips, np.stack(mats).astype(np.float32)


def _build_weights(inputs):
    """Returns wt [C, 93*C] fp32 (lhsT layout: wt[c, blk*C + o] = M_blk[o, c];
    blocks: 68 offset terms, 24 corrections, 1 y4-pointwise), k4 [C, 17],
    beff [C] fp32, offsets, k4_offsets, strips."""
    w1, w2, w3, w4 = inputs["w1"], inputs["w2"], inputs["w3"], inputs["w4"]
    b1, b2, b3, b4 = inputs["b1"], inputs["b2"], inputs["b3"], inputs["b4"]
    offsets, mats, k4_offsets, k4_mat, w4c = _build_terms(w1, w2, w3, w4)
    strips, cmats = _build_corrections(w2, w3, w4)
    allm = np.concatenate([mats, cmats, w4c[None]], axis=0)  # [93, C(o), C(c)]
    wt = np.ascontiguousarray(allm.transpose(2, 0, 1).reshape(C, -1)).astype(np.float32)
    w4m = w4[:, :, 0, 0].astype(np.float64)
    beff = (b4.astype(np.float64)
            + w4m @ (b1 + b2 + b3).astype(np.float64)).astype(np.float32)
    return wt, k4_mat, beff, offsets, k4_offsets, strips


# --------------------------------------------------------------------------
# device program
# --------------------------------------------------------------------------

_CACHE = {}


def _build_program(offsets, k4_offsets, strips):
    import concourse.bacc as bacc
    import concourse.mybir as mybir
    import concourse.tile as tile

    nc = bacc.Bacc("TRN2", target_bir_lowering=False)
    f32 = mybir.dt.float32
    f32r = mybir.dt.float32r

    bf16 = mybir.dt.bfloat16
    n_blk = 68 + 24 + 1
    Y4_BLK = 68 + 24
    # Weights are bf16: fp32/f32r weights force a self-loading matmul which
    # has a single sync-wait slot and breaks walrus codegen under Tile's
    # slot-reuse waits. bf16 weights use the standard LDWEIGHTS+MATMUL split.
    xp_d = nc.dram_tensor("xp", [C, HP * WP], bf16, kind="ExternalInput")
    wt_d = nc.dram_tensor("wt", [C, n_blk * C], bf16, kind="ExternalInput")
    xres_d = nc.dram_tensor("xres", [C, H * W], f32, kind="ExternalInput")
    k4_d = nc.dram_tensor("k4", [C, 17], f32, kind="ExternalInput")
    beff_d = nc.dram_tensor("beff", [C, 1], f32, kind="ExternalInput")
    out_d = nc.dram_tensor("out", [C, H * W], f32, kind="ExternalOutput")

    with tile.TileContext(nc) as tc:
        with (
            tc.tile_pool(name="const", bufs=1) as const,
            tc.tile_pool(name="outp", bufs=3) as outp,
            tc.tile_pool(name="psum", bufs=2, space="PSUM") as psum_pool,
            tc.tile_pool(name="y4p", bufs=2) as y4p,
        ):
            xp_sb = const.tile([C, HP * WP], bf16)
            wt_sb = const.tile([C, n_blk * C], bf16)
            xres_sb = const.tile([C, H * W], f32)
            k4_sb = const.tile([C, 17], f32)
            beff_sb = const.tile([C, 1], f32)

            # DMA strategy: SWDGE (nc.gpsimd) fans >=1MB transfers across all
            # 16 SDMA engines (~340 GB/s); HWDGE chunks ran at ~26 GB/s on a
            # single engine. Order: minimum needed for SB0 first (wt chunk 0 +
            # first xp rows), then the rest.
            WT_CHUNK = 55 * C
            nc.gpsimd.dma_start(out=wt_sb[:, :WT_CHUNK], in_=wt_d[:, :WT_CHUNK])
            ROWS0 = SB_ROWS + 2 * PAD  # padded rows needed by SB0
            nc.gpsimd.dma_start(out=xp_sb[:, :ROWS0 * WP],
                                in_=xp_d[:, :ROWS0 * WP])
            nc.gpsimd.dma_start(out=wt_sb[:, WT_CHUNK:], in_=wt_d[:, WT_CHUNK:])
            XP_CHUNK_ROWS = 56
            for r0_ in range(ROWS0, HP, XP_CHUNK_ROWS):
                r1_ = min(r0_ + XP_CHUNK_ROWS, HP)
                nc.gpsimd.dma_start(out=xp_sb[:, r0_ * WP:r1_ * WP],
                                    in_=xp_d[:, r0_ * WP:r1_ * WP])
            nc.sync.dma_start(out=beff_sb, in_=beff_d[:, :])
            nc.sync.dma_start(out=k4_sb, in_=k4_d[:, :])
            for q0 in range(0, H * W, H * W // 4):
                q1 = q0 + H * W // 4
                nc.gpsimd.dma_start(out=xres_sb[:, q0:q1], in_=xres_d[:, q0:q1])

            xp3 = xp_sb.rearrange("p (r w) -> p r w", w=WP)

            def wblk(i):
                return wt_sb[:, i * C:(i + 1) * C]

            corr_sb = const.tile([C, 8 * H], f32)

            def emit_corrections():
                # needs the full xp image -> emitted after SB0's matmuls
                psum_c_full = psum_pool.tile([C, SB_ROWS * W], f32, tag="acc",
                                             name="psum_c")
                psum_c = psum_c_full[:, :8 * H]
                for j, st in enumerate(strips):
                    for i, sh in enumerate(st["shifts"]):
                        if st["kind"] == "col":
                            # out rows 0..127 at fixed col; x col src, rows r+ty
                            rhs = xp3[:, PAD + sh: PAD + sh + H,
                                      PAD + st["src"]: PAD + st["src"] + 1]
                        else:
                            # out cols 0..127 at fixed row; x row src, cols j+tx
                            rhs = xp3[:, PAD + st["src"]: PAD + st["src"] + 1,
                                      PAD + sh: PAD + sh + W]
                        nc.tensor.matmul(psum_c[:, j * H:(j + 1) * H],
                                         wblk(68 + 3 * j + i), rhs,
                                         start=(i == 0), stop=(i == 2))
                nc.vector.tensor_copy(corr_sb, psum_c)

            # ---- main loop -------------------------------------------------
            n_sub = SB_ROWS // SUB_ROWS
            for s in range(N_SB):
                r0 = s * SB_ROWS
                # group-4 depthwise on the vector engine: 17 per-channel-scalar
                # FMA taps over the super-block (both coords of every offset
                # are even, so the bf16 reads stay 4B-aligned for 2x mode)
                y4 = y4p.tile([C, SB_ROWS * W], bf16)
                y43 = y4.rearrange("p (r w) -> p r w", w=W)
                for t, (dy, dx) in enumerate(k4_offsets):
                    xs = xp3[:, PAD + r0 + dy: PAD + r0 + dy + SB_ROWS,
                             PAD + dx: PAD + dx + W]
                    if t == 0:
                        nc.vector.tensor_scalar_mul(y4, xs, k4_sb[:, 0:1])
                    else:
                        nc.vector.scalar_tensor_tensor(
                            y4, xs, k4_sb[:, t:t + 1], y4,
                            mybir.AluOpType.mult, mybir.AluOpType.add)

                psum = psum_pool.tile([C, SB_ROWS * W], f32, tag="acc")
                for t, (di, dj) in enumerate(offsets):
                    for u in range(n_sub):
                        a0 = PAD + r0 + u * SUB_ROWS + di
                        rhs = xp3[:, a0: a0 + SUB_ROWS, PAD + dj: PAD + dj + W]
                        nc.tensor.matmul(
                            psum[:, u * SUB_ROWS * W:(u + 1) * SUB_ROWS * W],
                            wblk(t), rhs,
                            start=(t == 0), stop=False)
                for u in range(n_sub):
                    nc.tensor.matmul(
                        psum[:, u * SUB_ROWS * W:(u + 1) * SUB_ROWS * W],
                        wblk(Y4_BLK),
                        y43[:, u * SUB_ROWS:(u + 1) * SUB_ROWS, :],
                        start=False, stop=(u == n_sub - 1))

                if s == 0:
                    emit_corrections()

                out_sb = outp.tile([C, SB_ROWS * W], f32)
                nc.scalar.activation(out_sb, psum,
                                     mybir.ActivationFunctionType.Identity,
                                     bias=beff_sb[:, 0:1])
                nc.vector.tensor_add(
                    out_sb, out_sb,
                    xres_sb[:, r0 * W:(r0 + SB_ROWS) * W])
                out3 = out_sb.rearrange("p (r w) -> p r w", w=W)
                for j, st in enumerate(strips):
                    if st["kind"] == "col":
                        dst = out3[:, 0:SB_ROWS, st["fixed_out"]:st["fixed_out"] + 1]
                        src = corr_sb[:, j * H + r0: j * H + r0 + SB_ROWS]
                        nc.vector.tensor_add(dst, dst, src)
                    elif r0 <= st["fixed_out"] < r0 + SB_ROWS:
                        lr = st["fixed_out"] - r0
                        dst = out3[:, lr:lr + 1, :]
                        src = corr_sb[:, j * H: j * H + W]
                        nc.vector.tensor_add(dst, dst, src)
                nc.gpsimd.dma_start(out=out_d[:, r0 * W:(r0 + SB_ROWS) * W],
                                    in_=out_sb)
    nc.finalize()
    return nc


def _make_in_maps(inputs):
    x = np.ascontiguousarray(inputs["x"], dtype=np.float32)
    wt, k4_mat, beff, offsets, k4_offsets, strips = _build_weights(inputs)
    if "nc" not in _CACHE:
        _CACHE["nc"] = _build_program(offsets, k4_offsets, strips)

    import ml_dtypes
    bf = ml_dtypes.bfloat16
    xpad = np.zeros((B, C, HP, WP), bf)
    xpad[:, :, PAD:PAD + H, PAD:PAD + W] = x.astype(bf)
    beff_col = np.ascontiguousarray(beff.reshape(C, 1))
    wt_bf16 = wt.astype(bf)
    k4c = np.ascontiguousarray(k4_mat)
    return [
        {
            "xp": np.ascontiguousarray(xpad[b].reshape(C, HP * WP)),
            "wt": wt_bf16,
            "xres": np.ascontiguousarray(x[b].reshape(C, H * W)),
            "k4": k4c,
            "beff": beff_col,
        }
        for b in range(B)
    ]


def kernel(**inputs):
    in_maps = _make_in_maps(inputs)
    from concourse.bass_utils import run_bass_kernel_spmd
    res = run_bass_kernel_spmd(_CACHE["nc"], in_maps, core_ids=list(range(N_CORES)))
    out = np.stack([res.results[b]["out"].reshape(C, H, W) for b in range(B)])
    return out.astype(np.float32)
